# revision 1
# baseline (speedup 1.0000x reference)
"""ConceptNet encoder kernel for 8 Trainium2 NeuronCores (Bass/Tile).

Reference computation (see problem):
    emb    = table[tok]                      # [1024, 256]
    logits = emb @ table.T                   # [1024, 100000]
    idx    = top16(softmax(logits))          # softmax monotonic -> top16(logits)
    h      = table[idx]                      # [1024, 16, 256]
    e      = tanh(h @ a) @ b                 # [1024, 16]
    out    = softmax(e) @ h                  # [1024, 256]

Distribution: vocab (table rows) sharded 8 ways for the similarity matmul;
each core computes per-chunk top-8 candidates (max8 + max_index on DVE,
reading PSUM directly), then an AllToAll re-shards candidates by token so
each core merges + runs attention for its own 128 tokens.

kernel(**inputs) takes FULL unsharded inputs, returns FULL [4,256,256] output.
Self-contained: hardcodes all shapes; imports only the system concourse repo.
"""
import os
import sys

if "/opt/trn_rl_repo" not in sys.path:
    sys.path.insert(0, "/opt/trn_rl_repo")

import numpy as np

import concourse.bass as bass
import concourse.bacc as bacc
import concourse.mybir as mybir
import concourse.tile as tile
from concourse import bass_utils
from concourse.masks import make_identity

DT = mybir.dt

B, L, V, E, TOPK = 4, 256, 100000, 256, 16
NCORES = 8
NTOK = B * L                 # 1024
TPC = NTOK // NCORES         # 128 tokens per core (merge/attention shard)
VS = V // NCORES             # 12500 vocab rows per core
P = 128
NEG = -3.0e38

# similarity chunks within a core's 12500-column shard
CHUNK_W = 1024
CHUNKS = []
_off = 0
while _off < VS:
    CHUNKS.append((_off, min(CHUNK_W, VS - _off)))
    _off += CHUNK_W
NCHUNK = len(CHUNKS)          # 13 (12x1024 + 212)
NCAND = 128                   # 13*8 = 104 candidate slots, padded to 128
AGG_ELEMS = NCORES * 2 * TPC * NCAND   # flat fp32 elements in a2a output

_BUILD_CACHE = {}
LAST_RESULTS = None           # BassKernelResults of the most recent run


def _build(mm_dtype_name="float32", trace_names=False):
    mm_dt = getattr(DT, mm_dtype_name)
    nc = bacc.Bacc("TRN2", target_bir_lowering=False, debug=False,
                   enable_asserts=True, num_devices=NCORES)

    tokidx = nc.dram_tensor("tokidx", [NTOK, 1], DT.int32, kind="ExternalInput").ap()
    table = nc.dram_tensor("table", [V, E], DT.float32, kind="ExternalInput").ap()
    tabT = nc.dram_tensor("tabT", [E, VS], DT.float32, kind="ExternalInput").ap()
    amat = nc.dram_tensor("amat", [E, E], DT.float32, kind="ExternalInput").ap()
    bvec = nc.dram_tensor("bvec", [E, 1], DT.float32, kind="ExternalInput").ap()
    voff = nc.dram_tensor("voff", [1, 1], DT.uint32, kind="ExternalInput").ap()
    out = nc.dram_tensor("out", [TPC, E], DT.float32, kind="ExternalOutput").ap()

    with tile.TileContext(nc) as tc:
        with tc.tile_pool(name="const", bufs=1) as cpool, \
             tc.tile_pool(name="big", bufs=1) as big, \
             tc.tile_pool(name="work", bufs=2) as work, \
             tc.tile_pool(name="ps_chunk", bufs=3, space="PSUM") as ps_chunk, \
             tc.tile_pool(name="ps_tr", bufs=1, space="PSUM") as ps_tr, \
             tc.tile_pool(name="dram", bufs=1, space="DRAM") as dram:

            # ---------------- constants ----------------
            ident = cpool.tile([P, P], DT.float32, tag="ident")
            make_identity(nc, ident)

            # choff[p, s] = (s >> 3) * CHUNK_W  (vocab offset of chunk that
            # produced candidate slot s; 16 chunk slots x 8)
            choff = cpool.tile([P, NCAND], DT.uint32, tag="choff")
            nc.gpsimd.iota(choff, pattern=[[CHUNK_W, 16], [0, 8]], base=0,
                           channel_multiplier=0)

            # rowoff[p, 0] = NCAND*TPC + p*NCAND  (flat offset of the idx
            # plane's row p inside one source-core block of the a2a output)
            rowoff = cpool.tile([P, 1], DT.uint32, tag="rowoff")
            nc.gpsimd.iota(rowoff, pattern=[[0, 1]], base=TPC * NCAND,
                           channel_multiplier=NCAND)

            voff_sb = cpool.tile([1, 1], DT.uint32, tag="voff_sb")
            nc.sync.dma_start(out=voff_sb, in_=voff)
            voff_b = cpool.tile([P, 1], DT.uint32, tag="voff_b")
            nc.gpsimd.partition_broadcast(voff_b, voff_sb)

            # ---------------- resident weights ----------------
            tabT_sb = []
            for kb in range(2):
                t = big.tile([P, VS], mm_dt, tag=f"tabT{kb}")
                nc.sync.dma_start(out=t,
                                  in_=tabT[kb * P:(kb + 1) * P, :].bitcast(mm_dt))
                tabT_sb.append(t)

            a_sb = []
            for kb in range(2):
                t = cpool.tile([P, E], DT.float32, tag=f"a{kb}")
                nc.sync.dma_start(out=t, in_=amat[kb * P:(kb + 1) * P, :])
                a_sb.append(t)
            b_sb = []
            for kb in range(2):
                t = cpool.tile([P, 1], DT.float32, tag=f"b{kb}")
                nc.sync.dma_start(out=t, in_=bvec[kb * P:(kb + 1) * P, :])
                b_sb.append(t)

            # ---------------- emb gather + transpose ----------------
            embT = [big.tile([P, NTOK], mm_dt, tag=f"embT{kb}", name=f"embT{kb}")
                    for kb in range(2)]
            for m in range(NCORES):
                ti = work.tile([P, 1], DT.int32, tag="ti")
                nc.sync.dma_start(out=ti, in_=tokidx[m * P:(m + 1) * P, :])
                em = work.tile([P, E], DT.float32, tag="em")
                nc.gpsimd.indirect_dma_start(
                    out=em, out_offset=None, in_=table,
                    in_offset=bass.IndirectOffsetOnAxis(ap=ti[:, :], axis=0))
                for kb in range(2):
                    pt = ps_tr.tile([P, P], DT.float32, tag="tr")
                    nc.tensor.transpose(out=pt, in_=em[:, kb * P:(kb + 1) * P],
                                        identity=ident)
                    nc.vector.tensor_copy(embT[kb][:, m * P:(m + 1) * P], pt)

            # ---------------- a2a bounce buffers ----------------
            bounce = dram.tile([NCORES, 2, TPC, NCAND], DT.float32, tag="bounce")
            agg = dram.tile([AGG_ELEMS, 1], DT.float32, tag="agg")

            # ---------------- similarity + per-chunk top-8 ----------------
            for m in range(NCORES):
                cv = work.tile([P, NCAND], DT.float32, tag="cv")
                ci = work.tile([P, NCAND], DT.uint32, tag="ci")
                nc.vector.memset(cv, NEG)
                for j, (off, w) in enumerate(CHUNKS):
                    ps = ps_chunk.tile([P, CHUNK_W], DT.float32, tag="chunk")
                    for h in range((w + 511) // 512):
                        hw = min(512, w - h * 512)
                        for kb in range(2):
                            nc.tensor.matmul(
                                ps[:, h * 512:h * 512 + hw],
                                embT[kb][:, m * P:(m + 1) * P],
                                tabT_sb[kb][:, off + h * 512:off + h * 512 + hw],
                                start=(kb == 0), stop=(kb == 1))
                    nc.vector.max(out=cv[:, j * 8:(j + 1) * 8], in_=ps[:, :w])
                    nc.vector.max_index(out=ci[:, j * 8:(j + 1) * 8],
                                        in_max=cv[:, j * 8:(j + 1) * 8],
                                        in_values=ps[:, :w])
                # pad slots: candidate values already NEG; indices: whatever
                # max_index never wrote stays uninitialized -> memset pads
                nc.vector.memset(ci[:, NCHUNK * 8:], 0)
                # ci <- ci + choff + voff  (global vocab index)
                nc.vector.tensor_tensor(ci, ci, choff, op=mybir.AluOpType.add)
                nc.vector.tensor_tensor(ci, ci,
                                        voff_b[:, :].to_broadcast([P, NCAND]),
                                        op=mybir.AluOpType.add)
                nc.sync.dma_start(out=bounce[m, 0, :, :], in_=cv)
                nc.sync.dma_start(out=bounce[m, 1, :, :].bitcast(DT.uint32), in_=ci)

            # ---------------- AllToAll: reshard by token ----------------
            nc.gpsimd.collective_compute(
                "AllToAll", mybir.AluOpType.bypass,
                replica_groups=[list(range(NCORES))],
                ins=[bounce[:, :, :, :].opt()],
                outs=[agg[:, :].opt()],
            )

            # agg (flat) viewed as [src_core, plane, token_p, slot]
            agg_v = agg[:, :].rearrange("(a b p j) o -> a b p (j o)",
                                        a=NCORES, b=2, p=TPC)

            # ---------------- merge: global top-16 of 8*128 candidates ----
            vals = cpool.tile([P, NCORES * NCAND], DT.float32, tag="vals")
            for c in range(NCORES):
                nc.sync.dma_start(out=vals[:, c * NCAND:(c + 1) * NCAND],
                                  in_=agg_v[c, 0, :, :])
            wv = cpool.tile([P, TOPK], DT.float32, tag="wv")
            wpos = cpool.tile([P, TOPK], DT.uint32, tag="wpos")
            nc.vector.max(out=wv[:, 0:8], in_=vals)
            nc.vector.max_index(out=wpos[:, 0:8], in_max=wv[:, 0:8], in_values=vals)
            vals2 = cpool.tile([P, NCORES * NCAND], DT.float32, tag="vals2")
            nc.vector.match_replace(out=vals2, in_to_replace=wv[:, 0:8],
                                    in_values=vals, imm_value=NEG)
            nc.vector.max(out=wv[:, 8:16], in_=vals2)
            nc.vector.max_index(out=wpos[:, 8:16], in_max=wv[:, 8:16], in_values=vals2)

            # flat offsets into agg for the idx plane:
            #   off = (wpos>>7)*(2*TPC*NCAND) + TPC*NCAND + p*NCAND + (wpos&127)
            t1 = cpool.tile([P, TOPK], DT.uint32, tag="t1")
            t2 = cpool.tile([P, TOPK], DT.uint32, tag="t2")
            offs = cpool.tile([P, TOPK], DT.uint32, tag="offs")
            nc.vector.tensor_scalar(t1, wpos, 7, None,
                                    op0=mybir.AluOpType.logical_shift_right)
            nc.vector.tensor_scalar(t1, t1, 2 * TPC * NCAND, None,
                                    op0=mybir.AluOpType.mult)
            nc.vector.tensor_scalar(t2, wpos, NCAND - 1, None,
                                    op0=mybir.AluOpType.bitwise_and)
            nc.vector.tensor_tensor(offs, t1, t2, op=mybir.AluOpType.add)
            nc.vector.tensor_tensor(offs, offs,
                                    rowoff[:, :].to_broadcast([P, TOPK]),
                                    op=mybir.AluOpType.add)

            wgidx = cpool.tile([P, TOPK], DT.uint32, tag="wgidx")
            agg_u32 = agg[:, :].bitcast(DT.uint32)
            for k in range(TOPK):
                nc.gpsimd.indirect_dma_start(
                    out=wgidx[:, k:k + 1], out_offset=None, in_=agg_u32,
                    in_offset=bass.IndirectOffsetOnAxis(ap=offs[:, k:k + 1], axis=0))

            # ---------------- h gather ----------------
            h = cpool.tile([P, TOPK * E], DT.float32, tag="h")
            hv = h.rearrange("p (k e) -> p k e", k=TOPK)
            for k in range(TOPK):
                nc.gpsimd.indirect_dma_start(
                    out=hv[:, k, :], out_offset=None, in_=table,
                    in_offset=bass.IndirectOffsetOnAxis(ap=wgidx[:, k:k + 1], axis=0))

            # ---------------- attention pool ----------------
            # hT[kb][e, t*16+k] = h[t, k, kb*128+e]
            hT = [cpool.tile([P, TPC * TOPK], DT.float32, tag=f"hT{kb}", name=f"hT{kb}")
                  for kb in range(2)]
            for k in range(TOPK):
                for kb in range(2):
                    pt = ps_tr.tile([P, P], DT.float32, tag="tr")
                    nc.tensor.transpose(out=pt, in_=hv[:, k, kb * P:(kb + 1) * P],
                                        identity=ident)
                    dst = hT[kb].rearrange("e (t k) -> e t k", k=TOPK)[:, :, k]
                    nc.vector.tensor_copy(dst, pt)

            # tanh(h @ a)^T : [e', t*16+k]
            tanhT = [cpool.tile([P, TPC * TOPK], DT.float32, tag=f"tanhT{eb}", name=f"tanhT{eb}")
                     for eb in range(2)]
            NCH = (TPC * TOPK) // 512   # 4
            for eb in range(2):
                for n in range(NCH):
                    pt = ps_chunk.tile([P, 512], DT.float32, tag="chunk", name="att_ps")
                    for kb in range(2):
                        nc.tensor.matmul(pt, a_sb[kb][:, eb * P:(eb + 1) * P],
                                         hT[kb][:, n * 512:(n + 1) * 512],
                                         start=(kb == 0), stop=(kb == 1))
                    nc.scalar.activation(tanhT[eb][:, n * 512:(n + 1) * 512], pt,
                                         mybir.ActivationFunctionType.Tanh)

            # scores e[t,k] = tanh(...) @ b  -> [1, t*16+k]
            sc = cpool.tile([1, TPC * TOPK], DT.float32, tag="sc")
            for n in range(NCH):
                pt = ps_chunk.tile([1, 512], DT.float32, tag="chunk", name="sc_ps")
                for eb in range(2):
                    nc.tensor.matmul(pt, b_sb[eb], tanhT[eb][:, n * 512:(n + 1) * 512],
                                     start=(eb == 0), stop=(eb == 1))
                nc.vector.tensor_copy(sc[:, n * 512:(n + 1) * 512], pt)

            # reshape scores to [t, k] via DRAM roundtrip
            scd = dram.tile([1, TPC * TOPK], DT.float32, tag="scd")
            nc.sync.dma_start(out=scd, in_=sc)
            sct = cpool.tile([P, TOPK], DT.float32, tag="sct")
            nc.sync.dma_start(out=sct,
                              in_=scd[:, :].rearrange("o (t k) -> (o t) k", t=TPC))

            # softmax over k per token
            mx = cpool.tile([P, 1], DT.float32, tag="mx")
            nc.vector.reduce_max(mx, sct, axis=mybir.AxisListType.X)
            negmx = cpool.tile([P, 1], DT.float32, tag="negmx")
            nc.vector.tensor_scalar(negmx, mx, -1.0, None, op0=mybir.AluOpType.mult)
            ex = cpool.tile([P, TOPK], DT.float32, tag="ex")
            nc.scalar.activation(ex, sct, mybir.ActivationFunctionType.Exp,
                                 bias=negmx[:, :], scale=1.0)
            sm = cpool.tile([P, 1], DT.float32, tag="sm")
            nc.vector.reduce_sum(sm, ex, axis=mybir.AxisListType.X)
            rc = cpool.tile([P, 1], DT.float32, tag="rc")
            nc.vector.reciprocal(rc, sm)
            att = cpool.tile([P, TOPK], DT.float32, tag="att_w")
            nc.vector.tensor_scalar(att, ex, rc[:, :], None,
                                    op0=mybir.AluOpType.mult)

            # out[t, e] = sum_k att[t,k] * h[t,k,e]
            acc = cpool.tile([P, E], DT.float32, tag="acc")
            nc.vector.memset(acc, 0.0)
            for k in range(TOPK):
                term = work.tile([P, E], DT.float32, tag="term")
                nc.scalar.activation(term, hv[:, k, :],
                                     mybir.ActivationFunctionType.Copy,
                                     scale=att[:, k:k + 1])
                nc.vector.tensor_tensor(acc, acc, term, op=mybir.AluOpType.add)
            nc.sync.dma_start(out=out, in_=acc)

    nc.compile()
    return nc


def get_nc(mm_dtype_name=None):
    if mm_dtype_name is None:
        mm_dtype_name = os.environ.get("CN_MM_DT", "float32")
    if mm_dtype_name not in _BUILD_CACHE:
        _BUILD_CACHE[mm_dtype_name] = _build(mm_dtype_name)
    return _BUILD_CACHE[mm_dtype_name]


def kernel(conceptnet_text_vec, table, a, b, topk=16, **_ignored):
    global LAST_RESULTS
    assert int(topk) == TOPK
    tok = np.asarray(conceptnet_text_vec).reshape(NTOK, 1).astype(np.int32)
    table = np.ascontiguousarray(np.asarray(table, dtype=np.float32))
    a = np.ascontiguousarray(np.asarray(a, dtype=np.float32))
    b = np.ascontiguousarray(np.asarray(b, dtype=np.float32)).reshape(E, 1)
    tabT_full = np.ascontiguousarray(table.T)    # [E, V]

    nc = get_nc()
    in_maps = []
    for c in range(NCORES):
        in_maps.append({
            "tokidx": tok,
            "table": table,
            "tabT": np.ascontiguousarray(tabT_full[:, c * VS:(c + 1) * VS]),
            "amat": a,
            "bvec": b,
            "voff": np.full((1, 1), c * VS, np.uint32),
        })
    trace = bool(int(os.environ.get("CN_TRACE", "0")))
    res = bass_utils.run_bass_kernel_spmd(nc, in_maps, core_ids=list(range(NCORES)),
                                          trace=trace)
    LAST_RESULTS = res
    outp = np.concatenate([res.results[c]["out"] for c in range(NCORES)], axis=0)
    return outp.reshape(B, L, E)



# revision 6
# speedup vs baseline: 1.4316x; 1.4316x over previous
"""ConceptNet encoder kernel for 8 Trainium2 NeuronCores (Bass/Tile).

Reference computation:
    emb    = table[tok]                      # [1024, 256]
    logits = emb @ table.T                   # [1024, 100000]
    idx    = top16(softmax(logits))          # softmax monotonic -> top16(logits)
    h      = table[idx]                      # [1024, 16, 256]
    e      = tanh(h @ a) @ b                 # [1024, 16]
    out    = softmax(e) @ h                  # [1024, 256]

Distribution: vocab sharded 8 ways. Similarity matmul runs in float32r
(~bf16 speed, ~12-bit mantissa). Selection packs (quantized value, slot)
into one uint32 key per logit (scalar-engine quantize + one DVE
shift-or pass), takes per-chunk top-8 via a single MAX8 (no
FIND_INDEX8, no index plane), AllToAll's one key plane, merges a
top-24 candidate pool per token, re-scores the pool exactly in fp32
(rescue), and applies masked-softmax attention over the pool so only
the exact top-16 get weight.

kernel(**inputs) takes FULL unsharded inputs, returns FULL [4,256,256] output.
Self-contained: hardcodes all shapes; imports only the system concourse repo.
"""
import os
import sys

if "/opt/trn_rl_repo" not in sys.path:
    sys.path.insert(0, "/opt/trn_rl_repo")

import numpy as np

import concourse.bass as bass
import concourse.bacc as bacc
import concourse.mybir as mybir
import concourse.tile as tile
from concourse import bass_utils
from concourse.masks import make_identity

DT = mybir.dt
ALU = mybir.AluOpType
ACT = mybir.ActivationFunctionType

B, L, V, E, TOPK = 4, 256, 100000, 256, 16
NCORES = 8
NTOK = B * L                 # 1024
TPC = NTOK // NCORES         # 128 tokens per core (merge/attention shard)
VS = V // NCORES             # 12500 vocab rows per core
P = 128
NEG = -3.0e38

CW = 1024                    # similarity chunk width (2 PSUM banks)
CHUNKS = []
_off = 0
while _off < VS:
    CHUNKS.append((_off, min(CW, VS - _off)))
    _off += CW
NCHUNK = len(CHUNKS)         # 13 (12x1024 + 212)
NCAND = 128                  # per-(core,block) candidate slots: 72 real, padded
KP = 24                      # rescue pool size per token
KPAD = 32                    # padded pool for max8 rounds
QSCALE = 1536.0              # logit quantizer scale
QBIAS = 3456.0               # makes qi positive (logits in [-2.25, 2])
KEYSHIFT = 11                # slot bits (CW <= 2048)
KEYBASE = 1 << 30            # keeps key bit patterns in normal-float range
AGG_ELEMS = NCORES * TPC * NCAND

_BUILD_CACHE = {}
LAST_RESULTS = None


def _round12(x):
    """Round fp32 to 12 explicit mantissa bits (round half even)."""
    u = np.ascontiguousarray(x, dtype=np.float32).view(np.uint32)
    shift = np.uint32(11)
    mask = np.uint32((1 << 11) - 1)
    half = np.uint32(1 << 10)
    frac = u & mask
    u2 = u & ~mask
    rnd = (frac > half) | ((frac == half) & (((u2 >> shift) & np.uint32(1)) == 1))
    u2 = u2 + (rnd.astype(np.uint32) << shift)
    return u2.view(np.float32)


def _build():
    nc = bacc.Bacc("TRN2", target_bir_lowering=False, debug=False,
                   enable_asserts=True, num_devices=NCORES)

    tokidx = nc.dram_tensor("tokidx", [NTOK, 1], DT.int32, kind="ExternalInput").ap()
    tok_own = nc.dram_tensor("tok_own", [TPC, 1], DT.int32, kind="ExternalInput").ap()
    table = nc.dram_tensor("table", [V, E], DT.float32, kind="ExternalInput").ap()
    tabTr = nc.dram_tensor("tabTr", [E, VS], DT.float32r, kind="ExternalInput").ap()
    amat = nc.dram_tensor("amat", [E, E], DT.float32, kind="ExternalInput").ap()
    bvec = nc.dram_tensor("bvec", [E, 1], DT.float32, kind="ExternalInput").ap()
    out = nc.dram_tensor("out", [TPC, E], DT.float32, kind="ExternalOutput").ap()

    with tile.TileContext(nc) as tc:
        with tc.tile_pool(name="const", bufs=1) as cpool, \
             tc.tile_pool(name="big", bufs=1) as big, \
             tc.tile_pool(name="work", bufs=2) as work, \
             tc.tile_pool(name="ps_chunk", bufs=2, space="PSUM") as ps_chunk, \
             tc.tile_pool(name="ps_tr", bufs=2, space="PSUM") as ps_tr, \
             tc.tile_pool(name="ps_att", bufs=2, space="PSUM") as ps_att, \
             tc.tile_pool(name="dram", bufs=1, space="DRAM") as dram:

            # ---------------- constants ----------------
            ident = cpool.tile([P, P], DT.float32, tag="ident")
            make_identity(nc, ident)

            iotaK = cpool.tile([P, CW], DT.uint32, tag="iotaK")
            nc.gpsimd.iota(iotaK, pattern=[[1, CW]], base=KEYBASE,
                           channel_multiplier=0)

            def const_col(name, val):
                t = cpool.tile([P, 1], DT.uint32, tag=name)
                nc.gpsimd.iota(t, pattern=[[0, 1]], base=val, channel_multiplier=0)
                return t

            c_shift = const_col("c_shift", KEYSHIFT)
            c_slotmask = const_col("c_slotmask", (1 << KEYSHIFT) - 1)
            c_127 = const_col("c_127", 127)
            c_3 = const_col("c_3", 3)
            c_7 = const_col("c_7", 7)

            # ---------------- resident weights ----------------
            # tabTr split per (kb, chunk) so matmuls start as strips arrive.
            tabT_sb = [[None] * NCHUNK for _ in range(2)]
            for kb in range(2):
                for j, (off, w) in enumerate(CHUNKS):
                    t = big.tile([P, w], DT.float32r, tag=f"tt{kb}_{j}")
                    nc.sync.dma_start(out=t, in_=tabTr[kb * P:(kb + 1) * P,
                                                       off:off + w])
                    tabT_sb[kb][j] = t

            a_sb = []
            for kb in range(2):
                t = cpool.tile([P, E], DT.float32, tag=f"a{kb}")
                nc.sync.dma_start(out=t, in_=amat[kb * P:(kb + 1) * P, :])
                a_sb.append(t)
            a_r = []
            for kb in range(2):
                t = cpool.tile([P, E], DT.float32r, tag=f"ar{kb}")
                nc.vector.tensor_copy(t, a_sb[kb])
                a_r.append(t)
            b_sb = []
            for kb in range(2):
                t = cpool.tile([P, 1], DT.float32, tag=f"b{kb}")
                nc.sync.dma_start(out=t, in_=bvec[kb * P:(kb + 1) * P, :])
                b_sb.append(t)
            b_r = []
            for kb in range(2):
                t = cpool.tile([P, 1], DT.float32r, tag=f"br{kb}")
                nc.vector.tensor_copy(t, b_sb[kb])
                b_r.append(t)

            # ---------------- emb gather + f32r transpose ----------------
            embT = [big.tile([P, NTOK], DT.float32r, tag=f"embT{kb}",
                             name=f"embT{kb}")
                    for kb in range(2)]
            for m in range(NCORES):
                ti = work.tile([P, 1], DT.int32, tag="ti")
                nc.sync.dma_start(out=ti, in_=tokidx[m * P:(m + 1) * P, :])
                em = work.tile([P, E], DT.float32, tag="em")
                nc.gpsimd.indirect_dma_start(
                    out=em, out_offset=None, in_=table,
                    in_offset=bass.IndirectOffsetOnAxis(ap=ti[:, :], axis=0))
                for kb in range(2):
                    pt = ps_tr.tile([P, P], DT.float32, tag="tr")
                    nc.tensor.transpose(out=pt, in_=em[:, kb * P:(kb + 1) * P],
                                        identity=ident)
                    nc.vector.tensor_copy(embT[kb][:, m * P:(m + 1) * P], pt)

            # own-token embeddings (fp32, for exact rescue dots)
            ti_own = cpool.tile([P, 1], DT.int32, tag="ti_own")
            nc.sync.dma_start(out=ti_own, in_=tok_own)
            emb_own = cpool.tile([P, E], DT.float32, tag="emb_own")
            nc.gpsimd.indirect_dma_start(
                out=emb_own, out_offset=None, in_=table,
                in_offset=bass.IndirectOffsetOnAxis(ap=ti_own[:, :], axis=0))

            # ---------------- a2a bounce buffers ----------------
            bounce = dram.tile([NCORES, TPC, NCAND], DT.float32, tag="bounce")
            agg = dram.tile([AGG_ELEMS, 1], DT.float32, tag="agg")
            scd = dram.tile([1, TPC * KP], DT.float32, tag="scd")

            # ---------------- similarity + packed per-chunk top-8 --------
            for m in range(NCORES):
                cv = work.tile([P, NCAND], DT.float32, tag="cv")
                nc.vector.memset(cv[:, NCHUNK * 8:], 0.0)
                for j, (off, w) in enumerate(CHUNKS):
                    ps = ps_chunk.tile([P, CW], DT.float32, tag="chunk")
                    for kb in range(2):
                        for h in range((w + 511) // 512):
                            hw = min(512, w - h * 512)
                            nc.tensor.matmul(
                                ps[:, h * 512:h * 512 + hw],
                                embT[kb][:, m * P:(m + 1) * P],
                                tabT_sb[kb][j][:, h * 512:h * 512 + hw],
                                start=(kb == 0), stop=(kb == 1))
                    keys = work.tile([P, CW], DT.uint32, tag="keys")
                    # quantize logits -> int (scalar engine reads PSUM)
                    nc.scalar.activation(keys[:, :w].bitcast(DT.int32), ps[:, :w],
                                         ACT.Copy, scale=QSCALE, bias=QBIAS)
                    # key = (qi << 11) | slot | 2^30  (one DVE pass)
                    nc.vector.scalar_tensor_tensor(
                        keys[:, :w], keys[:, :w], c_shift[:, :], iotaK[:, :w],
                        op0=ALU.logical_shift_left, op1=ALU.bitwise_or)
                    nc.vector.max(out=cv[:, j * 8:(j + 1) * 8],
                                  in_=keys[:, :w].bitcast(DT.float32))
                nc.sync.dma_start(out=bounce[m, :, :], in_=cv)

            # ---------------- AllToAll: reshard by token ----------------
            nc.gpsimd.collective_compute(
                "AllToAll", ALU.bypass,
                replica_groups=[list(range(NCORES))],
                ins=[bounce[:, :, :].opt()],
                outs=[agg[:, :].opt()],
            )

            # vals[p, c*NCAND+s] = agg[(c, p, s)]
            vals = cpool.tile([P, NCORES * NCAND], DT.float32, tag="vals")
            agg_v = agg[:, :].rearrange("(c p s) o -> c p (s o)",
                                        c=NCORES, p=TPC)
            for c in range(NCORES):
                nc.sync.dma_start(out=vals[:, c * NCAND:(c + 1) * NCAND],
                                  in_=agg_v[c])

            # ---------------- merge: top-24 keys + positions -------------
            wk = cpool.tile([P, KPAD], DT.float32, tag="wk")
            wp = cpool.tile([P, KPAD], DT.uint32, tag="wp")
            vals2 = cpool.tile([P, NCORES * NCAND], DT.float32, tag="vals2")
            vals3 = cpool.tile([P, NCORES * NCAND], DT.float32, tag="vals3")
            nc.vector.max(out=wk[:, 0:8], in_=vals)
            nc.vector.max_index(out=wp[:, 0:8], in_max=wk[:, 0:8], in_values=vals)
            nc.vector.match_replace(out=vals2, in_to_replace=wk[:, 0:8],
                                    in_values=vals, imm_value=0.0)
            nc.vector.max(out=wk[:, 8:16], in_=vals2)
            nc.vector.max_index(out=wp[:, 8:16], in_max=wk[:, 8:16], in_values=vals2)
            nc.vector.match_replace(out=vals3, in_to_replace=wk[:, 8:16],
                                    in_values=vals2, imm_value=0.0)
            nc.vector.max(out=wk[:, 16:24], in_=vals3)
            nc.vector.max_index(out=wp[:, 16:24], in_max=wk[:, 16:24], in_values=vals3)

            # ---------------- decode global vocab indices ----------------
            # pos = c*128 + j*8 + r ; key = (qi<<11)|slot|2^30
            kp24 = slice(0, KP)
            slot = cpool.tile([P, KP], DT.uint32, tag="slot")
            nc.vector.tensor_scalar(slot, wk[:, kp24].bitcast(DT.uint32),
                                    c_slotmask[:, :], None, op0=ALU.bitwise_and)
            csrc = cpool.tile([P, KP], DT.uint32, tag="csrc")
            nc.vector.tensor_scalar(csrc, wp[:, kp24], c_7[:, :], None,
                                    op0=ALU.logical_shift_right)
            jchunk = cpool.tile([P, KP], DT.uint32, tag="jchunk")
            nc.vector.tensor_scalar(jchunk, wp[:, kp24], c_127[:, :], None,
                                    op0=ALU.bitwise_and)
            nc.vector.tensor_scalar(jchunk, jchunk, c_3[:, :], None,
                                    op0=ALU.logical_shift_right)
            # gidx = csrc*12500 + jchunk*1536 + slot  (all < 2^24: fp-exact)
            gidx = cpool.tile([P, KP], DT.uint32, tag="gidx")
            nc.vector.tensor_scalar(gidx, csrc, float(VS), None, op0=ALU.mult)
            t2 = cpool.tile([P, KP], DT.uint32, tag="t2")
            nc.vector.tensor_scalar(t2, jchunk, float(CW), None, op0=ALU.mult)
            nc.vector.tensor_tensor(gidx, gidx, t2, op=ALU.add)
            nc.vector.tensor_tensor(gidx, gidx, slot, op=ALU.add)

            # ---------------- rescue: gather h + exact fp32 dots ---------
            h = cpool.tile([P, KP * E], DT.float32, tag="h")
            hv = h.rearrange("p (k e) -> p k e", k=KP)
            gidx_i = gidx[:, :].bitcast(DT.int32)
            for k in range(KP):
                nc.gpsimd.indirect_dma_start(
                    out=hv[:, k, :], out_offset=None, in_=table,
                    in_offset=bass.IndirectOffsetOnAxis(ap=gidx_i[:, k:k + 1],
                                                        axis=0))
            d = cpool.tile([P, KPAD], DT.float32, tag="d")
            nc.vector.memset(d[:, KP:], NEG)
            prod = cpool.tile([P, E], DT.float32, tag="prod")
            for k in range(KP):
                nc.vector.scalar_tensor_tensor(
                    prod, hv[:, k, :], 1.0, emb_own,
                    op0=ALU.mult, op1=ALU.mult, accum_out=d[:, k:k + 1])

            # 16th largest exact dot -> threshold mask
            t8a = cpool.tile([P, 8], DT.float32, tag="t8a")
            t8b = cpool.tile([P, 8], DT.float32, tag="t8b")
            d2 = cpool.tile([P, KPAD], DT.float32, tag="d2")
            nc.vector.max(out=t8a, in_=d)
            nc.vector.match_replace(out=d2, in_to_replace=t8a, in_values=d,
                                    imm_value=NEG)
            nc.vector.max(out=t8b, in_=d2)
            # maskp = (1[d >= thr16] - 1) * 1e9   (0 for kept, -1e9 for dropped)
            maskp = cpool.tile([P, KP], DT.float32, tag="maskp")
            nc.vector.tensor_scalar(maskp, d[:, :KP], t8b[:, 7:8], None,
                                    op0=ALU.is_ge)
            nc.vector.tensor_scalar(maskp, maskp, -1.0, 1.0e9,
                                    op0=ALU.add, op1=ALU.mult)

            # ---------------- attention over the 24-candidate pool -------
            # hT chunks: n = k*128 + t, grouped 4 k's per 512-wide chunk
            NGR = KP // 4        # 6 groups
            for g in range(NGR):
                hTs = [work.tile([P, 512], DT.float32r, tag=f"hTs{kb}",
                                 name=f"hTs{kb}")
                       for kb in range(2)]
                for kk in range(4):
                    k = g * 4 + kk
                    for kb in range(2):
                        pt = ps_tr.tile([P, P], DT.float32, tag="tr")
                        nc.tensor.transpose(out=pt,
                                            in_=hv[:, k, kb * P:(kb + 1) * P],
                                            identity=ident)
                        nc.vector.tensor_copy(hTs[kb][:, kk * P:(kk + 1) * P], pt)
                tanhTs = [work.tile([P, 512], DT.float32r, tag=f"tanhTs{eb}",
                                    name=f"tanhTs{eb}")
                          for eb in range(2)]
                for eb in range(2):
                    pta = ps_att.tile([P, 512], DT.float32, tag="att")
                    for kb in range(2):
                        nc.tensor.matmul(pta, a_r[kb][:, eb * P:(eb + 1) * P],
                                         hTs[kb], start=(kb == 0), stop=(kb == 1))
                    nc.scalar.activation(tanhTs[eb], pta, ACT.Tanh)
                psc = ps_att.tile([1, 512], DT.float32, tag="att", name="psc")
                for eb in range(2):
                    nc.tensor.matmul(psc, b_r[eb], tanhTs[eb],
                                     start=(eb == 0), stop=(eb == 1))
                scs = work.tile([1, 512], DT.float32, tag="scs")
                nc.vector.tensor_copy(scs, psc)
                nc.sync.dma_start(out=scd[:, g * 512:(g + 1) * 512], in_=scs)

            # scores [t, k] <- scd[k*128 + t]
            sct = cpool.tile([P, KP], DT.float32, tag="sct")
            nc.sync.dma_start(out=sct,
                              in_=scd[:, :].rearrange("o (k t) -> (o t) k", t=TPC))

            # masked softmax over k
            nc.vector.tensor_tensor(sct, sct, maskp, op=ALU.add)
            mx = cpool.tile([P, 1], DT.float32, tag="mx")
            nc.vector.reduce_max(mx, sct, axis=mybir.AxisListType.X)
            negmx = cpool.tile([P, 1], DT.float32, tag="negmx")
            nc.vector.tensor_scalar(negmx, mx, -1.0, None, op0=ALU.mult)
            ex = cpool.tile([P, KP], DT.float32, tag="ex")
            nc.scalar.activation(ex, sct, ACT.Exp, bias=negmx[:, :], scale=1.0)
            sm = cpool.tile([P, 1], DT.float32, tag="sm")
            nc.vector.reduce_sum(sm, ex, axis=mybir.AxisListType.X)
            rc = cpool.tile([P, 1], DT.float32, tag="rc")
            nc.vector.reciprocal(rc, sm)
            att = cpool.tile([P, KP], DT.float32, tag="att_w")
            nc.vector.tensor_scalar(att, ex, rc[:, :], None, op0=ALU.mult)

            # out[t, e] = sum_k att[t,k] * h[t,k,e]
            acc = cpool.tile([P, E], DT.float32, tag="acc")
            nc.vector.memset(acc, 0.0)
            for k in range(KP):
                nc.vector.scalar_tensor_tensor(
                    acc, hv[:, k, :], att[:, k:k + 1], acc,
                    op0=ALU.mult, op1=ALU.add)
            nc.sync.dma_start(out=out, in_=acc)

    nc.compile()
    return nc


def get_nc():
    if "v2" not in _BUILD_CACHE:
        _BUILD_CACHE["v2"] = _build()
    return _BUILD_CACHE["v2"]


def kernel(conceptnet_text_vec, table, a, b, topk=16, **_ignored):
    global LAST_RESULTS
    assert int(topk) == TOPK
    tok = np.asarray(conceptnet_text_vec).reshape(NTOK, 1).astype(np.int32)
    table = np.ascontiguousarray(np.asarray(table, dtype=np.float32))
    a = np.ascontiguousarray(np.asarray(a, dtype=np.float32))
    b = np.ascontiguousarray(np.asarray(b, dtype=np.float32)).reshape(E, 1)
    tabT_r = _round12(np.ascontiguousarray(table.T))   # [E, V], f32r-rounded

    nc = get_nc()
    in_maps = []
    for c in range(NCORES):
        in_maps.append({
            "tokidx": tok,
            "tok_own": np.ascontiguousarray(tok[c * TPC:(c + 1) * TPC]),
            "table": table,
            "tabTr": np.ascontiguousarray(tabT_r[:, c * VS:(c + 1) * VS]),
            "amat": a,
            "bvec": b,
        })
    trace = bool(int(os.environ.get("CN_TRACE", "0")))
    res = bass_utils.run_bass_kernel_spmd(nc, in_maps, core_ids=list(range(NCORES)),
                                          trace=trace)
    LAST_RESULTS = res
    outp = np.concatenate([res.results[c]["out"] for c in range(NCORES)], axis=0)
    return outp.reshape(B, L, E)
